# revision 1
# baseline (speedup 1.0000x reference)
"""EquiformerV2 (2-layer) Bass/Tile kernel for 8 trn2 NeuronCores.

Sharding: dst-node-range parallel. Core c owns nodes [256c, 256c+256) and all
edges whose dst lands there. Per attention, each core computes its slice of
y_s = rms_norm(x)|restricted @ w_src, all-gathers it (the only large
collective), then gathers per-edge rows, computes messages / softmax / value
scatter fully locally via one-hot matmuls in PSUM.

On-chip layout: node state xT[g] is [128 channel, 128 nodes x 49 coeffs] per
node half-group g (channel-partitioned). Edge work runs in 128-edge tiles;
each half-group's edges occupy a fixed tile range (identical across cores, so
one SPMD program serves all 8 cores).
"""
import math
from contextlib import ExitStack

import numpy as np

import concourse.bass as bass
import concourse.bacc as bacc
import concourse.mybir as mybir
import concourse.tile as tile
from concourse.bass_utils import run_bass_kernel_spmd
from concourse.masks import make_identity

F32 = mybir.dt.float32
I32 = mybir.dt.int32
AF = mybir.ActivationFunctionType
ALU = mybir.AluOpType
AX = mybir.AxisListType

NCORES = 8
L_MAX, M_MAX = 6, 2
NC49 = (L_MAX + 1) ** 2
C = 128
H = 128
HEADS, VPH = 8, 16
FFN = 512
NB = 600
N, E, G = 2048, 12288, 16
NP = N // NCORES
AVG_DEG = 3.0
CUTOFF = 5.0
DISC_LO, DISC_HI = -3.26267, 3.295396
EPS = 1e-6

LBLK = [(l * l, 2 * l + 1) for l in range(L_MAX + 1)]
RBLK = []
_r = 0
for _l in range(L_MAX + 1):
    _cnt = min(2 * _l + 1, 2 * M_MAX + 1)
    RBLK.append((_r, _l * _l + _l - min(_l, M_MAX), _cnt))
    _r += _cnt
NR = _r                   # 29
W29 = NR * 128
W49 = NC49 * 128

_off_np = np.linspace(0.0, CUTOFF, NB).astype(np.float32)
GCOEF = float(-0.5 / (2.0 * (_off_np[1] - _off_np[0])) ** 2)
_mv_np = np.array([m for l in range(L_MAX + 1) for m in range(-l, l + 1)])
RESTRICT_NP = np.nonzero(np.abs(_mv_np) <= M_MAX)[0]


def real_sph_harm_np(vec):
    r = np.linalg.norm(vec, axis=-1, keepdims=True)
    u = vec / np.maximum(r, 1e-8)
    x, y, z = u[:, 0], u[:, 1], u[:, 2]
    ct = np.clip(z, -1.0, 1.0)
    st = np.sqrt(np.clip(1.0 - ct * ct, 1e-12, 1.0))
    phi = np.arctan2(y, x)
    P = {(0, 0): np.ones_like(ct)}
    for m in range(1, L_MAX + 1):
        P[(m, m)] = -(2 * m - 1) * st * P[(m - 1, m - 1)]
    for m in range(0, L_MAX):
        P[(m + 1, m)] = (2 * m + 1) * ct * P[(m, m)]
    for m in range(0, L_MAX + 1):
        for l in range(m + 2, L_MAX + 1):
            P[(l, m)] = ((2 * l - 1) * ct * P[(l - 1, m)] - (l + m - 1) * P[(l - 2, m)]) / (l - m)
    cols = []
    for l in range(L_MAX + 1):
        for m in range(-l, l + 1):
            am = abs(m)
            nrm = math.sqrt((2 * l + 1) / (4 * math.pi) * math.factorial(l - am) / math.factorial(l + am))
            if m == 0:
                cols.append(nrm * P[(l, 0)])
            elif m > 0:
                cols.append(math.sqrt(2.0) * nrm * P[(l, m)] * np.cos(m * phi))
            else:
                cols.append(math.sqrt(2.0) * nrm * P[(l, am)] * np.sin(am * phi))
    return np.stack(cols, axis=-1).astype(np.float32)


def host_prep(inputs):
    f = lambda k: np.asarray(inputs[k], np.float32)
    pos = f("pos")
    edge_vec = f("edge_vec")
    edge_index = np.asarray(inputs["edge_index"]).astype(np.int64)
    batch = np.asarray(inputs["batch"]).astype(np.int64)

    src, dst = edge_index[0], edge_index[1]
    d_all = np.linalg.norm(edge_vec, axis=-1).astype(np.float32)
    Y_all = (real_sph_harm_np(edge_vec) / np.float32(AVG_DEG)).astype(np.float32)

    t = np.clip(np.round((pos - DISC_LO) / (DISC_HI - DISC_LO) * 128.0 - 0.5), 0, 127).astype(np.int64)
    et_ = f("embed_table")
    emb = (et_[t[:, 0]] + et_[t[:, 1]] + et_[t[:, 2]]).astype(np.float32)

    core_of = dst // NP
    grp_of = (dst % NP) // 128
    lists = [[np.nonzero((core_of == c) & (grp_of == g))[0] for g in range(2)] for c in range(NCORES)]
    TG = max(1, (max(len(lists[c][g]) for c in range(NCORES) for g in range(2)) + 127) // 128)
    NT = 2 * TG
    EP = NT * 128

    cnt = np.bincount(batch, minlength=G).astype(np.float32)
    inv_cnt = (1.0 / np.maximum(cnt, 1.0)).astype(np.float32)

    nws = [f("attn_norm_w")[0], f("ffn_norm_w")[0], f("attn_norm_w")[1], f("ffn_norm_w")[1], f("final_norm_w")]
    nwT = np.concatenate([w.T for w in nws], axis=1).astype(np.float32)

    def stack_lat(key, lat_key):
        return np.concatenate([f(key)[0], f(key)[1], f(lat_key)], axis=0).astype(np.float32)

    w1b = np.concatenate([f("rad_w1"), f("rad_b1")[:, None, :]], axis=1)
    lw1b = np.concatenate([f("lat_rad_w1"), f("lat_rad_b1")[None, :]], axis=0)
    rad_w1b = np.concatenate([w1b[0], w1b[1], lw1b], axis=0).astype(np.float32)

    avec = np.stack([f("alpha_vec")[0].reshape(-1), f("alpha_vec")[1].reshape(-1),
                     f("lat_alpha").reshape(-1)], axis=0)
    avecR = np.repeat(avec[:, None, :], 128, axis=1).reshape(3 * 128, 128).astype(np.float32)

    deg_w1b = np.concatenate([f("deg_w1"), f("deg_b1")[None, :]], axis=0).astype(np.float32)
    deg_b2R = np.repeat(f("deg_b2")[None, :], 128, axis=0).astype(np.float32)

    offc = np.ascontiguousarray(_off_np.reshape(5, 120).T)

    shared = {
        "offc": offc, "nwT": nwT,
        "w_s": stack_lat("w_src", "lat_w_src"), "w_t": stack_lat("w_tgt", "lat_w_tgt"),
        "rad_w1b": rad_w1b, "rad_w2": stack_lat("rad_w2", "lat_rad_w2"),
        "avecR": avecR, "w_v": stack_lat("w_val", "lat_w_val"), "w_p": stack_lat("w_proj", "lat_w_proj"),
        "ffn_w1": np.concatenate([f("ffn_w1")[0], f("ffn_w1")[1]], axis=0).astype(np.float32),
        "ffn_w2": np.concatenate([f("ffn_w2")[0], f("ffn_w2")[1]], axis=0).astype(np.float32),
        "deg_w1b": deg_w1b, "deg_w2": f("deg_w2"), "deg_b2R": deg_b2R, "deg_w3": f("deg_w3"),
        "tick": np.zeros((1, 8), np.float32),
    }

    in_maps = []
    for c in range(NCORES):
        srcg = np.zeros((EP, 1), np.int32)
        dstg = np.zeros((EP, 1), np.int32)
        dstf = np.full((EP, 1), 1e9, np.float32)
        d_row = np.zeros((1, EP), np.float32)
        Yc = np.zeros((EP, NC49), np.float32)
        for g in range(2):
            idx = lists[c][g]
            o = g * TG * 128
            n = len(idx)
            srcg[o:o + n, 0] = src[idx]
            dstg[o:o + n, 0] = dst[idx] - c * NP
            dstf[o:o + n, 0] = (dst[idx] - c * NP).astype(np.float32)
            d_row[0, o:o + n] = d_all[idx]
            Yc[o:o + n] = Y_all[idx]
        embT = np.ascontiguousarray(emb[c * NP:(c + 1) * NP].T)
        PT = np.zeros((NP, G), np.float32)
        nloc = np.arange(c * NP, (c + 1) * NP)
        PT[np.arange(NP), batch[nloc]] = inv_cnt[batch[nloc]]
        m = dict(shared)
        m.update({"embT": embT, "d_row": d_row, "Y": Yc, "srcg": srcg, "dstg": dstg,
                  "dstf": dstf, "PT": PT})
        in_maps.append(m)
    return {"TG": TG, "NT": NT, "EP": EP}, in_maps


def _chunks(total, step=512):
    o = 0
    while o < total:
        yield o, min(step, total - o)
        o += step


def build_program(meta, debug=(), ablate=frozenset()):
    TG, NT, EP = meta["TG"], meta["NT"], meta["EP"]
    nc = bacc.Bacc("TRN2", target_bir_lowering=False, debug=False, num_devices=NCORES)

    def din(name, shape, dt=F32):
        return nc.dram_tensor(name, shape, dt, kind="ExternalInput")

    embT_d = din("embT", [128, NP])
    d_row_d = din("d_row", [1, EP])
    Y_d = din("Y", [EP, NC49])
    srcg_d = din("srcg", [EP, 1], I32)
    dstg_d = din("dstg", [EP, 1], I32)
    dstf_d = din("dstf", [EP, 1])
    PT_d = din("PT", [NP, G])
    offc_d = din("offc", [120, 5])
    nwT_d = din("nwT", [128, 35])
    w_s_d = din("w_s", [3 * 128, H])
    w_t_d = din("w_t", [3 * 128, H])
    rad_w1b_d = din("rad_w1b", [3 * 601, H])
    rad_w2_d = din("rad_w2", [3 * 128, H])
    avecR_d = din("avecR", [3 * 128, 128])
    w_v_d = din("w_v", [3 * 128, 128])
    w_p_d = din("w_p", [3 * 128, C])
    ffn_w1_d = din("ffn_w1", [2 * 128, FFN])
    ffn_w2_d = din("ffn_w2", [2 * FFN, C])
    deg_w1b_d = din("deg_w1b", [601, C])
    deg_w2_d = din("deg_w2", [C, C])
    deg_b2R_d = din("deg_b2R", [128, C])
    deg_w3_d = din("deg_w3", [C, (L_MAX + 1) * C])
    tick_d = din("tick", [1, 8])

    pooled_d = nc.dram_tensor("pooled", [G, W29], F32, kind="ExternalOutput")
    tock_d = nc.dram_tensor("tock", [1, 8], F32, kind="ExternalOutput")
    dbg_d = {name: nc.dram_tensor("dbg_" + name, list(shape), F32, kind="ExternalOutput")
             for name, shape in debug}

    BF = mybir.dt.bfloat16
    ys_loc = nc.dram_tensor("ys_loc", [NP, W29], BF)
    yt_loc = nc.dram_tensor("yt_loc", [NP, W29], BF)
    ys0_loc = nc.dram_tensor("ys0_loc", [NP, 128], F32)
    yt0_loc = nc.dram_tensor("yt0_loc", [NP, 128], F32)
    ys_full = nc.dram_tensor("ys_full", [N, W29], BF, addr_space="Shared")
    ys0_full = nc.dram_tensor("ys0_full", [N, 128], F32, addr_space="Shared")
    RG = [list(range(NCORES))]

    with tile.TileContext(nc) as tc, ExitStack() as es:
        per = es.enter_context(tc.tile_pool(name="persist", bufs=1))

        def dbg(name, ap):
            if name in dbg_d:
                nc.sync.dma_start(dbg_d[name][:], ap)

        ident = per.tile([128, 128], F32, tag="ident")
        make_identity(nc, ident[:])
        iota_i = per.tile([128, 128], I32, tag="iotai")
        nc.gpsimd.iota(iota_i[:], pattern=[[1, 128]], base=0, channel_multiplier=0)
        iota_f = per.tile([128, 128], F32, tag="iotaf")
        nc.vector.tensor_copy(iota_f[:], iota_i[:])
        ones1 = per.tile([1, 128], F32, tag="ones1")
        nc.vector.memset(ones1[:], 1.0)
        ones128 = per.tile([128, 128], F32, tag="ones128")
        nc.vector.memset(ones128[:], 1.0)

        xT = [per.tile([128, W49], F32, tag=f"xT{g}", name=f"xT{g}") for g in range(2)]
        rad_all = per.tile([128, 3 * NT * 128], F32, tag="rad_all")
        nwT = per.tile([128, 35], F32, tag="nwT")
        nc.sync.dma_start(nwT[:], nwT_d[:])
        embT = per.tile([128, NP], F32, tag="embT")
        nc.sync.dma_start(embT[:], embT_d[:])
        PT = [per.tile([128, G], F32, tag=f"PT{g}", name=f"PT{g}") for g in range(2)]
        for g in range(2):
            nc.sync.dma_start(PT[g][:], PT_d[g * 128:(g + 1) * 128, :])
        avecR, w_s, w_t, w_v, w_p, rad_w2 = [], [], [], [], [], []
        for a in range(3):
            sl = slice(a * 128, (a + 1) * 128)
            for lst, dram, tg in ((avecR, avecR_d, "av"), (w_s, w_s_d, "ws"), (w_t, w_t_d, "wt"),
                                  (w_v, w_v_d, "wv"), (w_p, w_p_d, "wp"), (rad_w2, rad_w2_d, "r2")):
                tl = per.tile([128, 128], F32, tag=f"{tg}{a}")
                nc.sync.dma_start(tl[:], dram[sl, :])
                lst.append(tl)
        ffn_w1, ffn_w2 = {}, {}
        for i in range(2):
            for fc in range(4):
                t1 = per.tile([128, 128], F32, tag=f"fw1_{i}_{fc}")
                nc.sync.dma_start(t1[:], ffn_w1_d[i * 128:(i + 1) * 128, fc * 128:(fc + 1) * 128])
                ffn_w1[(i, fc)] = t1
                t2 = per.tile([128, 128], F32, tag=f"fw2_{i}_{fc}")
                nc.sync.dma_start(t2[:], ffn_w2_d[i * FFN + fc * 128:i * FFN + (fc + 1) * 128, :])
                ffn_w2[(i, fc)] = t2

        for g in range(2):
            nc.vector.memset(xT[g][:], 0.0)
            nc.vector.tensor_copy(
                xT[g][:].rearrange("p (n k) -> p n k", k=NC49)[:, :, 0:1],
                embT[:, g * 128:(g + 1) * 128].rearrange("p n -> p n ()"))

        tkt = per.tile([1, 8], F32, tag="tkt")
        nc.sync.dma_start(tkt[:], tick_d[:])
        nc.scalar.add(tkt[:], tkt[:], 1.0)
        nc.sync.dma_start(tock_d[:], tkt[:])

        def build_S(et, g, dest_ap, pool):
            dstf_t = pool.tile([128, 1], F32, tag="dstf")
            nc.sync.dma_start(dstf_t[:], dstf_d[et * 128:(et + 1) * 128, :])
            dloc = pool.tile([128, 1], F32, tag="dloc")
            nc.vector.tensor_scalar_add(dloc[:], dstf_t[:], float(-128 * g))
            nc.vector.tensor_tensor(dest_ap, dloc[:].to_broadcast([128, 128]), iota_f[:],
                                    op=ALU.is_equal)

        def idx_tile(dram, et, pool, tg="idx"):
            t = pool.tile([128, 1], I32, tag=tg)
            nc.sync.dma_start(t[:], dram[et * 128:(et + 1) * 128, :])
            return t

        # ================ phase 0: smearing + radial MLPs + edge-degree ================
        skip_deg = "edgedeg" in ablate
        with tc.tile_pool(name="ph0", bufs=1) as ph0, tc.tile_pool(name="ph0s", bufs=2) as ph0s:
            offc = ph0.tile([120, 5], F32, tag="offc")
            nc.sync.dma_start(offc[:], offc_d[:])
            distT = ph0.tile([120, 5 * EP], F32, tag="distT")
            for ci in range(5):
                nc.sync.dma_start(distT[:, ci * EP:(ci + 1) * EP],
                                  d_row_d[0:1, :].to_broadcast([120, EP]))
            radD = ph0.tile([128, NT * 896], mybir.dt.bfloat16, tag="radD")
            degw1c, radw1c = [], {}
            for ci in range(5):
                wt = ph0.tile([120, C], F32, tag=f"degw1_{ci}")
                nc.sync.dma_start(wt[:], deg_w1b_d[ci * 120:(ci + 1) * 120, :])
                degw1c.append(wt)
                for a in range(3):
                    wr = ph0.tile([120, H], F32, tag=f"radw1_{a}_{ci}")
                    nc.sync.dma_start(wr[:], rad_w1b_d[a * 601 + ci * 120:a * 601 + (ci + 1) * 120, :])
                    radw1c[(a, ci)] = wr
            degb1 = ph0.tile([1, C], F32, tag="degb1")
            nc.sync.dma_start(degb1[:], deg_w1b_d[600:601, :])
            radb1 = []
            for a in range(3):
                rb = ph0.tile([1, H], F32, tag=f"radb1_{a}", name=f"radb1_{a}")
                nc.sync.dma_start(rb[:], rad_w1b_d[a * 601 + 600:a * 601 + 601, :])
                radb1.append(rb)
            degw1 = (degw1c, degb1)
            radw1 = {a: ([radw1c[(a, ci)] for ci in range(5)], radb1[a]) for a in range(3)}
            degw2 = ph0.tile([128, C], F32, tag="degw2")
            nc.sync.dma_start(degw2[:], deg_w2_d[:])
            degb2R = ph0.tile([128, C], F32, tag="degb2R")
            nc.sync.dma_start(degb2R[:], deg_b2R_d[:])
            degw3 = ph0.tile([128, 7 * C], F32, tag="degw3")
            nc.sync.dma_start(degw3[:], deg_w3_d[:])

            with tc.tile_pool(name="ph0p", bufs=2, space="PSUM") as ph0p:
                for ci in range(5):
                    sl = distT[:, ci * EP:(ci + 1) * EP]
                    nc.vector.tensor_scalar(sl, sl, offc[:, ci:ci + 1], None, op0=ALU.subtract)
                    nc.scalar.activation(sl, sl, AF.Square)
                    nc.scalar.activation(sl, sl, AF.Exp, scale=GCOEF)

                def mlp_front(et, wts_bias):
                    wts, bias = wts_bias
                    ps = ph0p.tile([128, 128], F32, tag="mlp1", space="PSUM")
                    for ci in range(5):
                        nc.tensor.matmul(ps[:],
                                         lhsT=distT[:, ci * EP + et * 128:ci * EP + (et + 1) * 128],
                                         rhs=wts[ci][:], start=(ci == 0), stop=False)
                    nc.tensor.matmul(ps[:], lhsT=ones1[:], rhs=bias[:], start=False, stop=True)
                    s1 = ph0s.tile([128, 128], F32, tag="s1")
                    nc.scalar.activation(s1[:], ps[:], AF.Silu)
                    return s1

                for et in ([] if skip_deg else range(NT)):
                    s1 = mlp_front(et, degw1)
                    pT = ph0p.tile([128, 128], F32, tag="pT", space="PSUM")
                    s1T = ph0s.tile([128, 128], F32, tag="s1T")
                    nc.tensor.transpose(pT[:], s1[:], ident[:])
                    nc.scalar.copy(s1T[:], pT[:])
                    ps2 = ph0p.tile([128, 128], F32, tag="mlp2", space="PSUM")
                    nc.tensor.matmul(ps2[:], lhsT=s1T[:], rhs=degw2[:], start=True, stop=True)
                    s2 = ph0s.tile([128, 128], F32, tag="s2")
                    nc.vector.tensor_add(s2[:], ps2[:], degb2R[:])
                    nc.scalar.activation(s2[:], s2[:], AF.Silu)
                    s2T = ph0s.tile([128, 128], F32, tag="s2T")
                    pT2 = ph0p.tile([128, 128], F32, tag="pT", space="PSUM")
                    nc.tensor.transpose(pT2[:], s2[:], ident[:])
                    nc.scalar.copy(s2T[:], pT2[:])
                    ps3 = ph0p.tile([128, 896], F32, tag="mlp3", space="PSUM", bufs=1)
                    for o, s in _chunks(7 * C):
                        nc.tensor.matmul(ps3[:, o:o + s], lhsT=s2T[:], rhs=degw3[:, o:o + s],
                                         start=True, stop=True)
                    nc.scalar.copy(radD[:, et * 896:(et + 1) * 896], ps3[:])
                    for a in range(3):
                        r1 = mlp_front(et, radw1[a])
                        r1T = ph0s.tile([128, 128], F32, tag="r1T")
                        pT3 = ph0p.tile([128, 128], F32, tag="pT", space="PSUM")
                        nc.tensor.transpose(pT3[:], r1[:], ident[:])
                        nc.scalar.copy(r1T[:], pT3[:])
                        psr = ph0p.tile([128, 128], F32, tag="mlp2", space="PSUM")
                        nc.tensor.matmul(psr[:], lhsT=r1T[:], rhs=rad_w2[a][:], start=True, stop=True)
                        nc.scalar.copy(rad_all[:, (a * NT + et) * 128:(a * NT + et + 1) * 128], psr[:])

            if "radD" in dbg_d:
                nc.gpsimd.dma_start(dbg_d["radD"][:], radD[:, 0:896])
            # edge-degree embedding scatter (S-stationary, n-partitioned SBUF accum)
            PASSES = [(0, 30), (30, NC49 - 30)]
            with tc.tile_pool(name="degp", bufs=1, space="PSUM") as degp:
                acc0 = degp.tile([128, 4096], F32, tag="acc", space="PSUM")
                deg_sb = ph0s.tile([128, 30 * 128], F32, tag="deg_sb", bufs=1)
                for g in ([] if skip_deg else range(2)):
                    for (k0, nk) in PASSES:
                        nc.vector.memset(deg_sb[:, 0:nk * 128], 0.0)
                        for ti in range(TG):
                            et = g * TG + ti
                            S = ph0s.tile([128, 128], F32, tag="S")
                            build_S(et, g, S[:], ph0s)
                            Yt = ph0s.tile([128, NC49], F32, tag="Yt")
                            nc.sync.dma_start(Yt[:], Y_d[et * 128:(et + 1) * 128, :])
                            M = ph0s.tile([128, 30 * 128], F32, tag="M", bufs=1)
                            for l in range(L_MAX + 1):
                                ks, kc = LBLK[l]
                                lo, hi = max(ks, k0), min(ks + kc, k0 + nk)
                                if lo >= hi:
                                    continue
                                nc.vector.tensor_tensor(
                                    M[:, (lo - k0) * 128:(hi - k0) * 128]
                                        .rearrange("p (k c) -> p k c", c=128),
                                    Yt[:, lo:hi].rearrange("p k -> p k ()").to_broadcast([128, hi - lo, 128]),
                                    radD[:, et * 896 + l * 128:et * 896 + (l + 1) * 128]
                                        .rearrange("p c -> p () c").to_broadcast([128, hi - lo, 128]),
                                    op=ALU.mult)
                            for o, s in _chunks(nk * 128):
                                nc.tensor.matmul(acc0[:, o:o + s], lhsT=S[:], rhs=M[:, o:o + s],
                                                 start=True, stop=True)
                            nc.vector.tensor_add(deg_sb[:, 0:nk * 128], deg_sb[:, 0:nk * 128],
                                                 acc0[:, 0:nk * 128])
                        # transpose each [n, c] block back to [c, n] and add into xT
                        for kb in range(nk):
                            k = k0 + kb
                            nc.tensor.transpose(acc0[:, 3584:3712],
                                                deg_sb[:, kb * 128:(kb + 1) * 128], ident[:])
                            xs = xT[g][:].rearrange("p (n k) -> p n k", k=NC49)[:, :, k:k + 1]
                            nc.vector.tensor_add(xs, xs,
                                                 acc0[:, 3584:3712].rearrange("p n -> p n ()"))
        dbg("xT0", xT[0][:])
        dbg("xT1", xT[1][:])

        # ---------- rms norm ----------
        def rms_norm(nidx, restricted, out_tiles, psp, sbp):
            stride = NR if restricted else NC49
            blocks = RBLK if restricted else [(ks, ks, kc) for (ks, kc) in LBLK]
            for g in range(2):
                sq = sbp.tile([128, W49], F32, tag="sq")
                nc.scalar.activation(sq[:], xT[g][:], AF.Square)
                red = sbp.tile([128, 7 * 128], F32, tag="nrm_red")
                for l in range(L_MAX + 1):
                    ks, kc = LBLK[l]
                    nc.vector.tensor_reduce(
                        red[:, l * 128:(l + 1) * 128],
                        sq[:].rearrange("p (n k) -> p n k", k=NC49)[:, :, ks:ks + kc],
                        axis=AX.X, op=ALU.add)
                ms = psp.tile([128, 7 * 128], F32, tag="nrm_ms", space="PSUM")
                for o, s in _chunks(7 * 128):
                    nc.tensor.matmul(ms[:, o:o + s], lhsT=ones128[:],
                                     rhs=red[:, o:o + s], start=True, stop=True)
                inv = sbp.tile([128, 7 * 128], F32, tag="nrm_inv")
                for l in range(L_MAX + 1):
                    nc.vector.tensor_scalar(inv[:, l * 128:(l + 1) * 128],
                                            ms[:, l * 128:(l + 1) * 128],
                                            float(1.0 / ((2 * l + 1) * C)), EPS,
                                            op0=ALU.mult, op1=ALU.add)
                nc.scalar.activation(inv[:], inv[:], AF.Sqrt)
                nc.vector.reciprocal(inv[:], inv[:])
                for l, (os_, ks, cnt) in enumerate(blocks):
                    ov = out_tiles[g][:].rearrange("p (n k) -> p n k", k=stride)[:, :, os_:os_ + cnt]
                    nc.vector.tensor_tensor(
                        ov,
                        xT[g][:].rearrange("p (n k) -> p n k", k=NC49)[:, :, ks:ks + cnt],
                        inv[:, l * 128:(l + 1) * 128].rearrange("p n -> p n ()")
                            .to_broadcast([128, 128, cnt]),
                        op=ALU.mult)
                    nc.scalar.activation(ov, ov, AF.Copy, scale=nwT[:, nidx * 7 + l:nidx * 7 + l + 1])

        # ---------- attention ----------
        def attention(a, nidx):
            last = (a == 2)
            esA = ExitStack()
            ap_ = esA.enter_context(tc.tile_pool(name=f"at{a}", bufs=1))
            S_all = ap_.tile([128, NT * 128], F32, tag="S_all")
            logits = ap_.tile([128, NT * 8], F32, tag="logits")
            exs = ap_.tile([128, NT * 8], F32, tag="exs")
            alpha8 = ap_.tile([128, NT * 8], F32, tag="alpha8")
            rden = ap_.tile([128, 16], F32, tag="rden")

            # --- norm + y + allgather ---
            with tc.tile_pool(name=f"at{a}n", bufs=1) as np_:
                hrT = [np_.tile([128, W29], F32, tag=f"hrT{g}", name=f"hrT{g}") for g in range(2)]
                with tc.tile_pool(name=f"at{a}nn", bufs=2, space="PSUM") as nrmp:
                    rms_norm(nidx, True, hrT, nrmp, np_)
                if a == 0:
                    dbg("hrT0", hrT0_dummy := hrT[0][:])
                with tc.tile_pool(name=f"at{a}np", bufs=2, space="PSUM") as npp:
                    for g in ([] if "yphase" in ablate else range(2)):
                        for (wt, loc, loc0) in ((w_s[a], ys_loc, ys0_loc), (w_t[a], yt_loc, yt0_loc)):
                            ysT = np_.tile([128, W29], F32, tag="ysT")
                            yacc = npp.tile([128, 4096], F32, tag="yacc", space="PSUM", bufs=1)
                            for o, s in _chunks(W29):
                                nc.tensor.matmul(yacc[:, o:o + s], lhsT=wt[:], rhs=hrT[g][:, o:o + s],
                                                 start=True, stop=True)
                            nc.scalar.copy(ysT[:], yacc[:, 0:W29])
                            if a == 0 and g == 0 and wt is w_t[0]:
                                dbg("ytT0", ysT[:])
                            ysrow = np_.tile([128, W29], F32, tag="ysrow")
                            for r0 in range(0, NR, 8):
                                nr = min(8, NR - r0)
                                for j in range(nr):
                                    nc.tensor.transpose(
                                        yacc[:, j * 512:j * 512 + 128],
                                        ysT[:].rearrange("p (n k) -> p k n", k=NR)[:, r0 + j, :],
                                        ident[:])
                                nc.scalar.copy(
                                    ysrow[:, r0 * 128:(r0 + nr) * 128].rearrange("p (j c) -> p j c", c=128),
                                    yacc[:].rearrange("p (j c) -> p j c", c=512)[:, 0:nr, 0:128])
                            nc.gpsimd.dma_start(loc[g * 128:(g + 1) * 128, :], ysrow[:])
                            nc.sync.dma_start(loc0[g * 128:(g + 1) * 128, :], ysrow[:, 0:128])
            if "ag" not in ablate:
                nc.gpsimd.collective_compute("AllGather", ALU.bypass, replica_groups=RG,
                                             ins=[ys_loc[:]], outs=[ys_full[:]])
                nc.gpsimd.collective_compute("AllGather", ALU.bypass, replica_groups=RG,
                                             ins=[ys0_loc[:]], outs=[ys0_full[:]])

            # --- pass 1: logits, softmax weights ---
            if "pass1" in ablate:
                nc.vector.memset(alpha8[:], 0.5)
                nc.vector.memset(S_all[:], 0.0)
            with tc.tile_pool(name=f"at{a}1", bufs=3) as p1, \
                 tc.tile_pool(name=f"at{a}1p", bufs=2, space="PSUM") as p1p:
                for et in ([] if "pass1" in ablate else range(NT)):
                    g = et // TG
                    build_S(et, g, S_all[:, et * 128:(et + 1) * 128], p1)
                    src_t = idx_tile(srcg_d, et, p1, "src")
                    dst_t = idx_tile(dstg_d, et, p1, "dst")
                    m0 = p1.tile([128, 128], F32, tag="m0")
                    nc.gpsimd.indirect_dma_start(out=m0[:], out_offset=None, in_=yt0_loc[:],
                                                 in_offset=bass.IndirectOffsetOnAxis(ap=dst_t[:, :1], axis=0))
                    nc.gpsimd.indirect_dma_start(out=m0[:], out_offset=None, in_=ys0_full[:],
                                                 in_offset=bass.IndirectOffsetOnAxis(ap=src_t[:, :1], axis=0),
                                                 compute_op=ALU.add)
                    nc.vector.tensor_mul(m0[:], m0[:],
                                         rad_all[:, (a * NT + et) * 128:(a * NT + et + 1) * 128])
                    nc.scalar.activation(m0[:], m0[:], AF.Silu)
                    nc.vector.tensor_mul(m0[:], m0[:], avecR[a][:])
                    nc.vector.tensor_reduce(logits[:, et * 8:(et + 1) * 8],
                                            m0[:].rearrange("p (h d) -> p h d", h=8),
                                            axis=AX.X, op=ALU.add)
                if "pass1" not in ablate:
                    # global max -> per-partition scalar
                    pmax = p1.tile([128, 1], F32, tag="pmax")
                    nc.vector.tensor_reduce(pmax[:], logits[:], axis=AX.X, op=ALU.max)
                    prow_p = p1p.tile([128, 128], F32, tag="small", space="PSUM")
                    nc.tensor.matmul(prow_p[0:1, :], lhsT=pmax[:], rhs=ident[:], start=True, stop=True)
                    prow = p1.tile([1, 128], F32, tag="prow")
                    nc.scalar.copy(prow[:], prow_p[0:1, :])
                    mx11 = p1.tile([1, 1], F32, tag="mx11")
                    nc.vector.tensor_reduce(mx11[:], prow[:], axis=AX.X, op=ALU.max)
                    mcol_p = p1p.tile([128, 128], F32, tag="small", space="PSUM")
                    nc.tensor.matmul(mcol_p[:, 0:1], lhsT=ones1[:], rhs=mx11[:], start=True, stop=True)
                    mxcol = p1.tile([128, 1], F32, tag="mxcol")
                    nc.scalar.copy(mxcol[:], mcol_p[:, 0:1])
                    nc.vector.tensor_scalar(exs[:], logits[:], mxcol[:, 0:1], None, op0=ALU.subtract)
                    nc.scalar.activation(exs[:], exs[:], AF.Exp)
                    # denominators per dst group
                    for g in range(2):
                        dps = p1p.tile([128, 8], F32, tag="den", space="PSUM")
                        for ti in range(TG):
                            et = g * TG + ti
                            nc.tensor.matmul(dps[:], lhsT=S_all[:, et * 128:(et + 1) * 128],
                                             rhs=exs[:, et * 8:(et + 1) * 8],
                                             start=(ti == 0), stop=(ti == TG - 1))
                        nc.vector.tensor_scalar_max(rden[:, g * 8:(g + 1) * 8], dps[:], 1e-9)
                        nc.vector.reciprocal(rden[:, g * 8:(g + 1) * 8], rden[:, g * 8:(g + 1) * 8])
                    for et in range(NT):
                        g = et // TG
                        stp = p1p.tile([128, 128], F32, tag="small", space="PSUM")
                        nc.tensor.transpose(stp[:], S_all[:, et * 128:(et + 1) * 128], ident[:])
                        STt = p1.tile([128, 128], F32, tag="STt")
                        nc.scalar.copy(STt[:], stp[:])
                        dep = p1p.tile([128, 8], F32, tag="de", space="PSUM")
                        nc.tensor.matmul(dep[:], lhsT=STt[:], rhs=rden[:, g * 8:(g + 1) * 8],
                                         start=True, stop=True)
                        nc.vector.tensor_mul(alpha8[:, et * 8:(et + 1) * 8],
                                             exs[:, et * 8:(et + 1) * 8], dep[:])
            if a == 0:
                dbg("logits0", logits[:])
                dbg("alpha0", alpha8[:])

            # --- pass 2: messages, values, scatter ---
            es2 = ExitStack()
            p2 = es2.enter_context(tc.tile_pool(name=f"at{a}2", bufs=2))
            p2p = es2.enter_context(tc.tile_pool(name=f"at{a}2p", bufs=1, space="PSUM"))
            acc = p2p.tile([128, 4096], F32, tag="acc", space="PSUM")
            agg = []
            lat = []
            VS, TS = 3712, 3840   # psum scratch offsets: v at 3712, transpose at 3840
            for g in ([] if "pass2" in ablate else range(2)):
                agn = p2.tile([128, W29], F32, tag="agn", name="agn", bufs=1)
                nc.vector.memset(agn[:], 0.0)
                for ti in range(TG):
                    et = g * TG + ti
                    src_t = idx_tile(srcg_d, et, p2, "src")
                    dst_t = idx_tile(dstg_d, et, p2, "dst")
                    msgb = p2.tile([128, W29], mybir.dt.bfloat16, tag="msgb", bufs=(1 if last else 2))
                    msg2 = p2.tile([128, W29], mybir.dt.bfloat16, tag="msg2b", bufs=(1 if last else 2))
                    nc.gpsimd.indirect_dma_start(out=msgb[:], out_offset=None, in_=yt_loc[:],
                                                 in_offset=bass.IndirectOffsetOnAxis(ap=dst_t[:, :1], axis=0))
                    nc.gpsimd.indirect_dma_start(out=msg2[:], out_offset=None, in_=ys_full[:],
                                                 in_offset=bass.IndirectOffsetOnAxis(ap=src_t[:, :1], axis=0))
                    msg = p2.tile([128, W29], F32, tag="msg", bufs=1)
                    nc.vector.tensor_add(msgb[:], msgb[:], msg2[:])
                    nc.vector.tensor_tensor(
                        msg[:].rearrange("p (r h) -> p r h", h=128),
                        msgb[:].rearrange("p (r h) -> p r h", h=128),
                        rad_all[:, (a * NT + et) * 128:(a * NT + et + 1) * 128]
                            .rearrange("p h -> p () h").to_broadcast([128, NR, 128]),
                        op=ALU.mult)
                    if a == 0 and et == 0:
                        dbg("msg00", msg[:])
                    vsb = p2.tile([128, W29], F32, tag="vsb", bufs=(1 if last else 2))
                    mtt = p2.tile([128, W29], F32, tag="agg", name="mtt", bufs=1)
                    # transpose rounds: 8 blocks into 8 distinct banks, one batched copy
                    for r0 in range(0, NR, 8):
                        nr = min(8, NR - r0)
                        for j in range(nr):
                            nc.tensor.transpose(acc[:, j * 512:j * 512 + 128],
                                                msg[:, (r0 + j) * 128:(r0 + j + 1) * 128], ident[:])
                        nc.scalar.copy(
                            mtt[:, r0 * 128:(r0 + nr) * 128].rearrange("p (j c) -> p j c", c=128),
                            acc[:].rearrange("p (j c) -> p j c", c=512)[:, 0:nr, 0:128])
                    # value matmul rounds
                    for r0 in range(0, NR, 8):
                        nr = min(8, NR - r0)
                        for j in range(nr):
                            nc.tensor.matmul(acc[:, j * 512 + 128:j * 512 + 256],
                                             lhsT=mtt[:, (r0 + j) * 128:(r0 + j + 1) * 128],
                                             rhs=w_v[a][:], start=True, stop=True)
                        nc.vector.tensor_copy(
                            vsb[:, r0 * 128:(r0 + nr) * 128].rearrange("p (j c) -> p j c", c=128),
                            acc[:].rearrange("p (j c) -> p j c", c=512)[:, 0:nr, 128:256])
                    nc.vector.tensor_tensor(
                        vsb[:].rearrange("p (r h d) -> p r h d", r=NR, h=8),
                        vsb[:].rearrange("p (r h d) -> p r h d", r=NR, h=8),
                        alpha8[:, et * 8:(et + 1) * 8].rearrange("p h -> p () h ()")
                            .to_broadcast([128, NR, 8, 16]),
                        op=ALU.mult)
                    if a == 0 and et == 0:
                        dbg("vsb00", vsb[:])
                    for o, s in _chunks(W29):
                        nc.tensor.matmul(acc[:, o:o + s],
                                         lhsT=S_all[:, et * 128:(et + 1) * 128],
                                         rhs=vsb[:, o:o + s], start=True, stop=True)
                        nc.vector.tensor_add(agn[:, o:o + s], agn[:, o:o + s], acc[:, o:o + s])
                if last:
                    # per-r: transpose agn block, project with w_p, keep node-partitioned
                    lat_g = p2.tile([128, W29], F32, tag=f"lat{g}", name="lat_g", bufs=1)
                    lat.append(lat_g)
                    for r in range(NR):
                        nc.tensor.transpose(acc[:, TS:TS + 128], agn[:, r * 128:(r + 1) * 128], ident[:])
                        mt2 = p2.tile([128, 128], F32, tag="mt")
                        nc.scalar.copy(mt2[:], acc[:, TS:TS + 128])
                        nc.tensor.matmul(acc[:, VS:VS + 128], lhsT=mt2[:], rhs=w_p[a][:],
                                         start=True, stop=True)
                        nc.scalar.copy(lat_g[:, r * 128:(r + 1) * 128], acc[:, VS:VS + 128])
                    continue
                # agn is [128n, (r,vo)]; transpose blocks to [vo, n] form aggT
                ag = p2.tile([128, W29], F32, tag="agg", name="ag", bufs=1)
                for r0 in range(0, NR, 8):
                    nr = min(8, NR - r0)
                    for j in range(nr):
                        nc.tensor.transpose(acc[:, j * 512:j * 512 + 128],
                                            agn[:, (r0 + j) * 128:(r0 + j + 1) * 128], ident[:])
                    nc.scalar.copy(
                        ag[:, r0 * 128:(r0 + nr) * 128].rearrange("p (j c) -> p j c", c=128),
                        acc[:].rearrange("p (j c) -> p j c", c=512)[:, 0:nr, 0:128])
                if a == 0 and g == 0:
                    dbg("agg00", ag[:])
                agg.append(ag)
                # project + residual for this group
                for o, s in _chunks(W29):
                    nc.tensor.matmul(acc[:, o:o + s], lhsT=w_p[a][:], rhs=ag[:, o:o + s],
                                     start=True, stop=True)
                for (os_, ks, cnt) in RBLK:
                    xv = xT[g][:].rearrange("p (n k) -> p n k", k=NC49)[:, :, ks:ks + cnt]
                    nc.vector.tensor_add(
                        xv, xv,
                        acc[:, 0:W29].rearrange("p (r n) -> p n r", n=128)[:, :, os_:os_ + cnt])
            if last and "pass2" in ablate:
                pooled_sb = p2.tile([16, W29], F32, tag="agn", name="pooled_sb", bufs=1)
                nc.vector.memset(pooled_sb[:], 0.0)
                nc.sync.dma_start(pooled_d[:], pooled_sb[:])
            if last and "pass2" not in ablate:
                pooled_sb = p2.tile([16, W29], F32, tag="agn", name="pooled_sb", bufs=1)
                for o, s in _chunks(W29):
                    for g in range(2):
                        nc.tensor.matmul(acc[0:16, 0:s], lhsT=PT[g][:], rhs=lat[g][:, o:o + s],
                                         start=(g == 0), stop=(g == 1))
                    nc.scalar.copy(pooled_sb[:, o:o + s], acc[0:16, 0:s])
                nc.sync.dma_start(pooled_d[:], pooled_sb[:])
            es2.close()
            esA.close()

        # ---------- ffn ----------
        def ffn(i, nidx):
            if "ffn" in ablate:
                return
            with tc.tile_pool(name=f"ff{i}", bufs=1) as fp:
                hfull = [fp.tile([128, W49], F32, tag=f"hf{g}", name=f"hf{g}") for g in range(2)]
                with tc.tile_pool(name=f"ff{i}np", bufs=1, space="PSUM") as fnp:
                    rms_norm(nidx, False, hfull, fnp, fp)
                with tc.tile_pool(name=f"ff{i}p", bufs=1, space="PSUM") as ffp, \
                     tc.tile_pool(name=f"ff{i}s", bufs=2) as fs:
                    QW = 32 * NC49      # 1568 cols per quarter
                    for g in range(2):
                        for q in range(4):
                            hsl = hfull[g][:, q * QW:(q + 1) * QW]
                            ops = ffp.tile([128, QW], F32, tag="ops", space="PSUM")
                            for fc in range(4):
                                h1p = ffp.tile([128, QW], F32, tag="h1p", space="PSUM")
                                for o, s in _chunks(QW):
                                    nc.tensor.matmul(h1p[:, o:o + s], lhsT=ffn_w1[(i, fc)][:],
                                                     rhs=hsl[:, o:o + s], start=True, stop=True)
                                s_sl = h1p[:].rearrange("p (n k) -> p n k", k=NC49)[:, :, 0:1]
                                sg = fs.tile([128, 32], F32, tag="sg")
                                nc.scalar.activation(sg[:], s_sl.rearrange("p n k -> p (n k)"), AF.Sigmoid)
                                sl_ = fs.tile([128, 32], F32, tag="sl")
                                nc.scalar.activation(sl_[:], s_sl.rearrange("p n k -> p (n k)"), AF.Silu)
                                h1g = fs.tile([128, QW], F32, tag="h1g")
                                nc.vector.tensor_tensor(
                                    h1g[:].rearrange("p (n k) -> p n k", k=NC49),
                                    h1p[:].rearrange("p (n k) -> p n k", k=NC49),
                                    sg[:].rearrange("p n -> p n ()").to_broadcast([128, 32, NC49]),
                                    op=ALU.mult)
                                nc.vector.tensor_copy(
                                    h1g[:].rearrange("p (n k) -> p n k", k=NC49)[:, :, 0:1],
                                    sl_[:].rearrange("p n -> p n ()"))
                                for o, s in _chunks(QW):
                                    nc.tensor.matmul(ops[:, o:o + s], lhsT=ffn_w2[(i, fc)][:],
                                                     rhs=h1g[:, o:o + s], start=(fc == 0), stop=(fc == 3))
                            xsl = xT[g][:, q * QW:(q + 1) * QW]
                            nc.vector.tensor_add(xsl, xsl, ops[:])

        attention(0, 0)
        dbg("xT0_a0", xT[0][:])
        ffn(0, 1)
        dbg("xT0_f0", xT[0][:])
        attention(1, 2)
        ffn(1, 3)
        dbg("xT0_l1", xT[0][:])
        dbg("xT1_l1", xT[1][:])
        attention(2, 4)


    nc.compile()
    return nc


_CACHE = {}


def _get_program(meta, debug=()):
    key = (meta["TG"], tuple(n for n, _ in debug), tuple(sorted(ABLATE)))
    if key not in _CACHE:
        _CACHE[key] = build_program(meta, debug, frozenset(ABLATE))
    return _CACHE[key]


DEBUG_OUTS = ()
ABLATE = set()   # test.py may set e.g. (("xT0", [128, W49]), ...)


class _Runner:
    """Caches the jitted shard_map callable for a compiled program."""

    def __init__(self, nc):
        import jax
        from jax.sharding import Mesh, PartitionSpec
        from jax.experimental.shard_map import shard_map
        from concourse.bass2jax import _bass_exec_p, install_neuronx_cc_hook, partition_id_tensor
        install_neuronx_cc_hook()
        self.jax = jax
        pname = nc.partition_id_tensor.name if nc.partition_id_tensor else None
        in_names, out_names, out_avals, zeros = [], [], [], []
        for alloc in nc.m.functions[0].allocations:
            if not isinstance(alloc, mybir.MemoryLocationSet):
                continue
            name = alloc.memorylocations[0].name
            if alloc.kind == "ExternalInput":
                if name != pname:
                    in_names.append(name)
            elif alloc.kind == "ExternalOutput":
                out_names.append(name)
                shp = tuple(alloc.tensor_shape)
                dt = mybir.dt.np(alloc.dtype)
                out_avals.append(jax.core.ShapedArray(shp, dt))
                zeros.append(np.zeros((NCORES * shp[0],) + shp[1:], dt))
        self.in_names, self.out_names, self.zeros = in_names, out_names, zeros
        n_params, n_outs = len(in_names), len(out_names)
        names_all = in_names + out_names + ([pname] if pname else [])

        def _body(*args):
            operands = list(args)
            if pname is not None:
                operands.append(partition_id_tensor())
            return tuple(_bass_exec_p.bind(
                *operands, out_avals=tuple(out_avals), in_names=tuple(names_all),
                out_names=tuple(out_names), lowering_input_output_aliases=(),
                sim_require_finite=True, sim_require_nnan=True, nc=nc))

        devices = jax.devices()[:NCORES]
        self.mesh = Mesh(np.asarray(devices), ("core",))
        self.fn = jax.jit(shard_map(
            _body, mesh=self.mesh,
            in_specs=(PartitionSpec("core"),) * (n_params + n_outs),
            out_specs=(PartitionSpec("core"),) * n_outs, check_rep=False),
            keep_unused=True)

    def stage(self, in_maps):
        from jax.sharding import NamedSharding, PartitionSpec
        sh = NamedSharding(self.mesh, PartitionSpec("core"))
        args = [np.concatenate([np.asarray(m[n]) for m in in_maps], axis=0)
                for n in self.in_names] + list(self.zeros)
        return [self.jax.device_put(a, sh) for a in args]

    def __call__(self, staged):
        return self.fn(*staged)

    def results(self, outs):
        res = [dict() for _ in range(NCORES)]
        for i, n in enumerate(self.out_names):
            arr = np.asarray(outs[i])
            per = arr.reshape(NCORES, arr.shape[0] // NCORES, *arr.shape[1:])
            for c in range(NCORES):
                res[c][n] = per[c]
        return res


_RUNNERS = {}


def get_runner(meta, debug=()):
    key = (meta["TG"], tuple(n for n, _ in debug))
    if key not in _RUNNERS:
        _RUNNERS[key] = _Runner(_get_program(meta, debug))
    return _RUNNERS[key]


def kernel(**inputs):
    meta, in_maps = host_prep(inputs)
    runner = get_runner(meta, DEBUG_OUTS)
    staged = runner.stage(in_maps)
    outs = runner(staged)
    self_results = runner.results(outs)
    pooled = np.zeros((G, NR, 128), np.float32)
    for c in range(NCORES):
        pooled += self_results[c]["pooled"].reshape(G, NR, 128)
    out = np.zeros((G, NC49, C), np.float32)
    out[:, RESTRICT_NP, :] = pooled
    kernel.last_results = self_results
    kernel.last_runner = runner
    kernel.last_staged = staged
    return out.reshape(1, -1)



# revision 66
# speedup vs baseline: 3.0195x; 3.0195x over previous
"""EquiformerV2 (2-layer) Bass/Tile kernel for 8 trn2 NeuronCores — v2.

Sharding: dst-node-range parallel (core c owns nodes [256c, 256c+256) and the
edges terminating there). Per attention: y_s/y_t computed locally in bf16, one
AllGather of y_s, then per 128-edge tile a single gathered message tile feeds
logits, values and the one-hot-matmul scatter (PSUM-accumulated over tiles).

v2 vs baseline: all matmuls bf16 (4x PE), scatters accumulate in PSUM instead
of SBUF adds, gathers merged (one per side per tile), collective overlapped
with local gathers + radial MLPs, activation-LUT switches minimized.
"""
import math
from contextlib import ExitStack

import numpy as np

import concourse.bass as bass
import concourse.bacc as bacc
import concourse.mybir as mybir
import concourse.tile as tile
from concourse.bass_utils import run_bass_kernel_spmd
from concourse.masks import make_identity

F32 = mybir.dt.float32
BF = mybir.dt.bfloat16
I32 = mybir.dt.int32
AF = mybir.ActivationFunctionType
ALU = mybir.AluOpType
AX = mybir.AxisListType
BF_NP = mybir.dt.np(BF)

NCORES = 8
L_MAX, M_MAX = 6, 2
NC49 = (L_MAX + 1) ** 2
C = 128
H = 128
HEADS, VPH = 8, 16
FFN = 512
NB = 600
N, E, G = 2048, 12288, 16
NP = N // NCORES
AVG_DEG = 3.0
CUTOFF = 5.0
DISC_LO, DISC_HI = -3.26267, 3.295396
EPS = 1e-6

LBLK = [(l * l, 2 * l + 1) for l in range(L_MAX + 1)]
RBLK = []
_r = 0
for _l in range(L_MAX + 1):
    _cnt = min(2 * _l + 1, 2 * M_MAX + 1)
    RBLK.append((_r, _l * _l + _l - min(_l, M_MAX), _cnt))
    _r += _cnt
NR = _r                   # 29
W29 = NR * 128
W49 = NC49 * 128

_off_np = np.linspace(0.0, CUTOFF, NB).astype(np.float32)
GCOEF = float(-0.5 / (2.0 * (_off_np[1] - _off_np[0])) ** 2)
_mv_np = np.array([m for l in range(L_MAX + 1) for m in range(-l, l + 1)])
_deg_np = np.array([l for l in range(L_MAX + 1) for m in range(-l, l + 1)])
RESTRICT_NP = np.nonzero(np.abs(_mv_np) <= M_MAX)[0]


def real_sph_harm_np(vec):
    r = np.linalg.norm(vec, axis=-1, keepdims=True)
    u = vec / np.maximum(r, 1e-8)
    x, y, z = u[:, 0], u[:, 1], u[:, 2]
    ct = np.clip(z, -1.0, 1.0)
    st = np.sqrt(np.clip(1.0 - ct * ct, 1e-12, 1.0))
    phi = np.arctan2(y, x)
    P = {(0, 0): np.ones_like(ct)}
    for m in range(1, L_MAX + 1):
        P[(m, m)] = -(2 * m - 1) * st * P[(m - 1, m - 1)]
    for m in range(0, L_MAX):
        P[(m + 1, m)] = (2 * m + 1) * ct * P[(m, m)]
    for m in range(0, L_MAX + 1):
        for l in range(m + 2, L_MAX + 1):
            P[(l, m)] = ((2 * l - 1) * ct * P[(l - 1, m)] - (l + m - 1) * P[(l - 2, m)]) / (l - m)
    cols = []
    for l in range(L_MAX + 1):
        for m in range(-l, l + 1):
            am = abs(m)
            nrm = math.sqrt((2 * l + 1) / (4 * math.pi) * math.factorial(l - am) / math.factorial(l + am))
            if m == 0:
                cols.append(nrm * P[(l, 0)])
            elif m > 0:
                cols.append(math.sqrt(2.0) * nrm * P[(l, m)] * np.cos(m * phi))
            else:
                cols.append(math.sqrt(2.0) * nrm * P[(l, am)] * np.sin(am * phi))
    return np.stack(cols, axis=-1).astype(np.float32)


def host_prep(inputs):
    f = lambda k: np.asarray(inputs[k], np.float32)
    b = lambda a: np.ascontiguousarray(np.asarray(a, np.float32)).astype(BF_NP)
    pos = f("pos")
    edge_vec = f("edge_vec")
    edge_index = np.asarray(inputs["edge_index"]).astype(np.int64)
    batch = np.asarray(inputs["batch"]).astype(np.int64)

    src, dst = edge_index[0], edge_index[1]
    d_all = np.linalg.norm(edge_vec, axis=-1).astype(np.float32)
    Y_all = (real_sph_harm_np(edge_vec) / np.float32(AVG_DEG)).astype(np.float32)

    t = np.clip(np.round((pos - DISC_LO) / (DISC_HI - DISC_LO) * 128.0 - 0.5), 0, 127).astype(np.int64)
    et_ = f("embed_table")
    emb = (et_[t[:, 0]] + et_[t[:, 1]] + et_[t[:, 2]]).astype(np.float32)

    core_of = dst // NP
    grp_of = (dst % NP) // 128
    lists = [[np.nonzero((core_of == c) & (grp_of == g))[0] for g in range(2)] for c in range(NCORES)]
    TG = max(1, (max(len(lists[c][g]) for c in range(NCORES) for g in range(2)) + 127) // 128)
    NT = 2 * TG
    EP = NT * 128

    cnt = np.bincount(batch, minlength=G).astype(np.float32)
    inv_cnt = (1.0 / np.maximum(cnt, 1.0)).astype(np.float32)

    nws = [f("attn_norm_w")[0], f("ffn_norm_w")[0], f("attn_norm_w")[1], f("ffn_norm_w")[1], f("final_norm_w")]
    nwT = np.concatenate([w.T for w in nws], axis=1).astype(np.float32)

    def stack_lat(key, lat_key):
        return np.concatenate([f(key)[0], f(key)[1], f(lat_key)], axis=0).astype(np.float32)

    w1b = np.concatenate([f("rad_w1"), f("rad_b1")[:, None, :]], axis=1)
    lw1b = np.concatenate([f("lat_rad_w1"), f("lat_rad_b1")[None, :]], axis=0)
    rad_w1b = np.concatenate([w1b[0], w1b[1], lw1b], axis=0).astype(np.float32)

    avecC = np.stack([f("alpha_vec")[0].reshape(-1), f("alpha_vec")[1].reshape(-1),
                      f("lat_alpha").reshape(-1)], axis=1).astype(np.float32)  # [128, 3]
    Hsel = np.zeros((128, HEADS), np.float32)
    Hsel[np.arange(128), np.arange(128) // VPH] = 1.0

    deg_w1b = np.concatenate([f("deg_w1"), f("deg_b1")[None, :]], axis=0).astype(np.float32)

    offc = np.ascontiguousarray(_off_np.reshape(5, 120).T)

    shared = {
        "offc": offc, "nwT": nwT, "avecC": avecC, "Hsel": b(Hsel),
        "w_s": b(stack_lat("w_src", "lat_w_src")), "w_t": b(stack_lat("w_tgt", "lat_w_tgt")),
        "rad_w1b": b(rad_w1b), "rad_w2": b(stack_lat("rad_w2", "lat_rad_w2")),
        "w_v": b(stack_lat("w_val", "lat_w_val")), "w_p": b(stack_lat("w_proj", "lat_w_proj")),
        "ffn_w1": b(np.concatenate([f("ffn_w1")[0], f("ffn_w1")[1]], axis=0)),
        "ffn_w2": b(np.concatenate([f("ffn_w2")[0], f("ffn_w2")[1]], axis=0)),
        "deg_w1b": b(deg_w1b), "deg_w2": b(f("deg_w2")),
        "deg_b2r": b(f("deg_b2")[None, :]), "deg_w3": b(f("deg_w3")),
        "tick": np.zeros((1, 8), np.float32),
    }

    in_maps = []
    for c in range(NCORES):
        srcg = np.zeros((EP, 1), np.int32)
        dstg = np.zeros((EP, 1), np.int32)
        dstf = np.full((EP, 1), 1e9, np.float32)
        d_row = np.zeros((1, EP), np.float32)
        Yc = np.zeros((EP, NC49), np.float32)
        for g in range(2):
            idx = lists[c][g]
            o = g * TG * 128
            n = len(idx)
            srcg[o:o + n, 0] = src[idx]
            dstg[o:o + n, 0] = dst[idx] - c * NP
            dstf[o:o + n, 0] = (dst[idx] - c * NP).astype(np.float32)
            d_row[0, o:o + n] = d_all[idx]
            Yc[o:o + n] = Y_all[idx]
        embT = np.ascontiguousarray(emb[c * NP:(c + 1) * NP].T)
        PT = np.zeros((NP, G), np.float32)
        nloc = np.arange(c * NP, (c + 1) * NP)
        PT[np.arange(NP), batch[nloc]] = inv_cnt[batch[nloc]]
        YtT = np.ascontiguousarray(Yc.reshape(NT, 128, NC49).transpose(1, 0, 2).reshape(128, NT * NC49))
        m = dict(shared)
        m.update({"embT": embT, "d_row": d_row, "Yt": b(YtT),
                  "srcT": np.ascontiguousarray(srcg.reshape(NT, 128).T),
                  "dstT": np.ascontiguousarray(dstg.reshape(NT, 128).T),
                  "dstf": dstf, "PT": b(PT)})
        in_maps.append(m)
    return {"TG": TG, "NT": NT, "EP": EP}, in_maps


def _chunks(total, step=512):
    o = 0
    while o < total:
        yield o, min(step, total - o)
        o += step


def build_program(meta, debug=(), ablate=frozenset()):
    TG, NT, EP = meta["TG"], meta["NT"], meta["EP"]
    nc = bacc.Bacc("TRN2", target_bir_lowering=False, debug=False, num_devices=NCORES)

    def din(name, shape, dt=F32):
        return nc.dram_tensor(name, shape, dt, kind="ExternalInput")

    embT_d = din("embT", [128, NP])
    d_row_d = din("d_row", [1, EP])
    Yt_d = din("Yt", [128, NT * NC49], BF)
    srcT_d = din("srcT", [128, NT], I32)
    dstT_d = din("dstT", [128, NT], I32)
    dstf_d = din("dstf", [EP, 1])
    PT_d = din("PT", [NP, G], BF)
    offc_d = din("offc", [120, 5])
    nwT_d = din("nwT", [128, 35])
    avecC_d = din("avecC", [128, 3])
    Hsel_d = din("Hsel", [128, HEADS], BF)
    w_s_d = din("w_s", [3 * 128, H], BF)
    w_t_d = din("w_t", [3 * 128, H], BF)
    rad_w1b_d = din("rad_w1b", [3 * 601, H], BF)
    rad_w2_d = din("rad_w2", [3 * 128, H], BF)
    w_v_d = din("w_v", [3 * 128, 128], BF)
    w_p_d = din("w_p", [3 * 128, C], BF)
    ffn_w1_d = din("ffn_w1", [2 * 128, FFN], BF)
    ffn_w2_d = din("ffn_w2", [2 * FFN, C], BF)
    deg_w1b_d = din("deg_w1b", [601, C], BF)
    deg_w2_d = din("deg_w2", [C, C], BF)
    deg_b2r_d = din("deg_b2r", [1, C], BF)
    deg_w3_d = din("deg_w3", [C, (L_MAX + 1) * C], BF)
    tick_d = din("tick", [1, 8])

    pooled_d = nc.dram_tensor("pooled", [G, W29], BF, kind="ExternalOutput")
    tock_d = nc.dram_tensor("tock", [1, 8], F32, kind="ExternalOutput")
    dbg_d = {name: nc.dram_tensor("dbg_" + name, list(shape), F32, kind="ExternalOutput")
             for name, shape in debug}

    ys_loc = nc.dram_tensor("ys_loc", [NP, W29], BF)
    yt_loc = nc.dram_tensor("yt_loc", [NP, W29], BF)
    ys_full = nc.dram_tensor("ys_full", [N, W29], BF, addr_space="Shared")
    RG = [list(range(NCORES))]

    with tile.TileContext(nc) as tc, ExitStack() as es:
        per = es.enter_context(tc.tile_pool(name="persist", bufs=1))

        def dbg(name, ap):
            if name in dbg_d:
                if ap.dtype != F32:
                    nc.gpsimd.dma_start(dbg_d[name][:], ap)
                else:
                    nc.sync.dma_start(dbg_d[name][:], ap)

        ident = per.tile([128, 128], F32, tag="ident")
        make_identity(nc, ident[:])
        ident_b = per.tile([128, 128], BF, tag="identb")
        nc.vector.tensor_copy(ident_b[:], ident[:])
        ones1 = per.tile([1, 128], BF, tag="ones1")
        nc.vector.memset(ones1[:], 1.0)
        ones1f = per.tile([1, 128], F32, tag="ones1f")
        nc.vector.memset(ones1f[:], 1.0)
        ones128b = per.tile([128, 128], BF, tag="ones128")
        nc.vector.memset(ones128b[:], 1.0)

        xT = [per.tile([128, W49], F32, tag=f"xT{g}", name=f"xT{g}") for g in range(2)]
        rad_all = per.tile([128, 3 * NT * 128], BF, tag="rad_all")
        S_all = per.tile([128, NT * 128], BF, tag="S_all")
        idx_src = per.tile([128, NT], I32, tag="idx_src")
        nc.sync.dma_start(idx_src[:], srcT_d[:])
        idx_dst = per.tile([128, NT], I32, tag="idx_dst")
        nc.sync.dma_start(idx_dst[:], dstT_d[:])
        nwT = per.tile([128, 35], F32, tag="nwT")
        nc.sync.dma_start(nwT[:], nwT_d[:])
        avecC = per.tile([128, 3], F32, tag="avecC")
        nc.sync.dma_start(avecC[:], avecC_d[:])
        Hsel = per.tile([128, HEADS], BF, tag="Hsel")
        nc.sync.dma_start(Hsel[:], Hsel_d[:])
        PT = [per.tile([128, G], BF, tag=f"PT{g}", name=f"PT{g}") for g in range(2)]
        for g in range(2):
            nc.sync.dma_start(PT[g][:], PT_d[g * 128:(g + 1) * 128, :])
        w_s, w_t, w_v, w_p, rad_w2 = [], [], [], [], []
        for a in range(3):
            sl = slice(a * 128, (a + 1) * 128)
            for lst, dram, tg in ((w_s, w_s_d, "ws"), (w_t, w_t_d, "wt"),
                                  (w_v, w_v_d, "wv"), (w_p, w_p_d, "wp"), (rad_w2, rad_w2_d, "r2")):
                tl = per.tile([128, 128], BF, tag=f"{tg}{a}")
                nc.sync.dma_start(tl[:], dram[sl, :])
                lst.append(tl)
        ffn_w1, ffn_w2 = {}, {}
        for i in range(2):
            for fc in range(4):
                t1 = per.tile([128, 128], BF, tag=f"fw1_{i}_{fc}")
                nc.sync.dma_start(t1[:], ffn_w1_d[i * 128:(i + 1) * 128, fc * 128:(fc + 1) * 128])
                ffn_w1[(i, fc)] = t1
                t2 = per.tile([128, 128], BF, tag=f"fw2_{i}_{fc}")
                nc.sync.dma_start(t2[:], ffn_w2_d[i * FFN + fc * 128:i * FFN + (fc + 1) * 128, :])
                ffn_w2[(i, fc)] = t2
        # dist MLP weight chunks (shared across the whole run)
        distT = per.tile([120, 5 * EP], BF, tag="distT")
        degw1c, radw1c = [], {}
        for ci in range(5):
            wt = per.tile([120, C], BF, tag=f"degw1_{ci}")
            nc.sync.dma_start(wt[:], deg_w1b_d[ci * 120:(ci + 1) * 120, :])
            degw1c.append(wt)
            for a in range(3):
                wr = per.tile([120, H], BF, tag=f"radw1_{a}_{ci}")
                nc.sync.dma_start(wr[:], rad_w1b_d[a * 601 + ci * 120:a * 601 + (ci + 1) * 120, :])
                radw1c[(a, ci)] = wr
        degb1 = per.tile([1, C], BF, tag="degb1")
        nc.sync.dma_start(degb1[:], deg_w1b_d[600:601, :])
        degb2 = per.tile([1, C], BF, tag="degb2")
        nc.sync.dma_start(degb2[:], deg_b2r_d[:])
        radb1 = []
        for a in range(3):
            rb = per.tile([1, H], BF, tag=f"radb1_{a}", name=f"radb1_{a}")
            nc.sync.dma_start(rb[:], rad_w1b_d[a * 601 + 600:a * 601 + 601, :])
            radb1.append(rb)
        degw2 = per.tile([128, C], BF, tag="degw2")
        nc.sync.dma_start(degw2[:], deg_w2_d[:])
        degw3 = per.tile([128, 7 * C], BF, tag="degw3")
        nc.sync.dma_start(degw3[:], deg_w3_d[:])

        tkt = per.tile([1, 8], F32, tag="tkt")
        nc.sync.dma_start(tkt[:], tick_d[:])
        nc.scalar.add(tkt[:], tkt[:], 1.0)
        nc.sync.dma_start(tock_d[:], tkt[:])

        copy_rr = [nc.scalar, nc.vector]

        def copy_eng(i, out_ap, in_ap):
            e = copy_rr[i % len(copy_rr)]
            if e is nc.scalar:
                e.copy(out_ap, in_ap)
            else:
                e.tensor_copy(out_ap, in_ap)

        # ---------------- phase A: distT, S/ST, radD + edge-degree ----------------
        with tc.tile_pool(name="ph0", bufs=1) as ph0, \
             tc.tile_pool(name="ph0s", bufs=2) as ph0s:
            iota_i = ph0.tile([128, 128], I32, tag="iotai")
            nc.gpsimd.iota(iota_i[:], pattern=[[1, 128]], base=0, channel_multiplier=0)
            iota_f = ph0.tile([128, 128], F32, tag="iotaf")
            nc.vector.tensor_copy(iota_f[:], iota_i[:])
            embT = ph0.tile([128, NP], F32, tag="embT")
            nc.sync.dma_start(embT[:], embT_d[:])
            for g in range(2):
                nc.gpsimd.memset(xT[g][:], 0.0)
                nc.vector.tensor_copy(
                    xT[g][:].rearrange("p (n k) -> p n k", k=NC49)[:, :, 0:1],
                    embT[:, g * 128:(g + 1) * 128].rearrange("p n -> p n ()"))
            offc = ph0.tile([120, 5], F32, tag="offc")
            nc.sync.dma_start(offc[:], offc_d[:])
            distF = ph0.tile([120, 5 * EP], F32, tag="distF")
            for ci in range(5):
                nc.sync.dma_start(distF[:, ci * EP:(ci + 1) * EP],
                                  d_row_d[0:1, :].to_broadcast([120, EP]))
                nc.vector.tensor_scalar(distF[:, ci * EP:(ci + 1) * EP],
                                        distF[:, ci * EP:(ci + 1) * EP],
                                        offc[:, ci:ci + 1], None, op0=ALU.subtract)
            nc.scalar.activation(distF[:], distF[:], AF.Square)
            nc.scalar.activation(distT[:], distF[:], AF.Exp, scale=GCOEF)

            # S (edge->node one-hot), shared by all phases
            for et in range(NT):
                g = et // TG
                dstf_t = ph0s.tile([128, 1], F32, tag="dstf")
                nc.sync.dma_start(dstf_t[:], dstf_d[et * 128:(et + 1) * 128, :])
                dloc = ph0s.tile([128, 1], F32, tag="dloc")
                nc.vector.tensor_scalar_add(dloc[:], dstf_t[:], float(-128 * g))
                nc.vector.tensor_tensor(S_all[:, et * 128:(et + 1) * 128],
                                        dloc[:].to_broadcast([128, 128]), iota_f[:],
                                        op=ALU.is_equal)

            # radD: edge-degree MLP (silu LUT set throughout)
            radD = ph0.tile([128, NT * 896], BF, tag="radD")
            Yt_all = ph0.tile([128, NT * NC49], BF, tag="Yt_all")
            nc.sync.dma_start(Yt_all[:], Yt_d[:])

            skip_deg = "edgedeg" in ablate
            with tc.tile_pool(name="ph0p", bufs=1, space="PSUM") as ph0p:
                for et in ([] if skip_deg else range(NT)):
                    ps = ph0p.tile([128, 128], F32, tag="mlp1", space="PSUM", bufs=2)
                    for ci in range(5):
                        nc.tensor.matmul(ps[:],
                                         lhsT=degw1c[ci][:],
                                         rhs=distT[:, ci * EP + et * 128:ci * EP + (et + 1) * 128],
                                         start=(ci == 0), stop=False)
                    nc.tensor.matmul(ps[:], lhsT=degb1[:], rhs=ones1[:], start=False, stop=True)
                    s1T = ph0s.tile([128, 128], BF, tag="s1T")
                    nc.scalar.activation(s1T[:], ps[:], AF.Silu)
                    ps2 = ph0p.tile([128, 128], F32, tag="mlp2", space="PSUM", bufs=2)
                    nc.tensor.matmul(ps2[:], lhsT=degw2[:], rhs=s1T[:], start=True, stop=False)
                    nc.tensor.matmul(ps2[:], lhsT=degb2[:], rhs=ones1[:], start=False, stop=True)
                    s2T = ph0s.tile([128, 128], BF, tag="s2T")
                    nc.scalar.activation(s2T[:], ps2[:], AF.Silu)
                    ps3 = ph0p.tile([128, 896], F32, tag="mlp3", space="PSUM", bufs=1)
                    for o, s in _chunks(7 * C):
                        nc.tensor.matmul(ps3[:, o:o + s], lhsT=s2T[:], rhs=degw3[:, o:o + s],
                                         start=True, stop=True)
                    nc.vector.tensor_copy(radD[:, et * 896:(et + 1) * 896], ps3[:])

            # edge-degree scatter: PSUM-accumulated over tiles, per 8-coeff chunk
            with tc.tile_pool(name="degp", bufs=2, space="PSUM") as degp, \
                 tc.tile_pool(name="degt", bufs=2, space="PSUM") as degt, \
                 tc.tile_pool(name="degs", bufs=2) as degs:
                for g in ([] if skip_deg else range(2)):
                    for k0 in range(0, NC49, 8):
                        nk = min(8, NC49 - k0)
                        acc = degp.tile([128, 1024], F32, tag="dacc", space="PSUM")
                        for ti in range(TG):
                            et = g * TG + ti
                            M = degs.tile([128, 1024], BF, tag="M")
                            for l in range(L_MAX + 1):
                                ks, kc = LBLK[l]
                                lo, hi = max(ks, k0), min(ks + kc, k0 + nk)
                                if lo >= hi:
                                    continue
                                nc.gpsimd.tensor_tensor(
                                    M[:, (lo - k0) * 128:(hi - k0) * 128]
                                        .rearrange("p (k c) -> p k c", c=128),
                                    Yt_all[:, et * NC49 + lo:et * NC49 + hi]
                                        .rearrange("p k -> p k ()").to_broadcast([128, hi - lo, 128]),
                                    radD[:, et * 896 + l * 128:et * 896 + (l + 1) * 128]
                                        .rearrange("p c -> p () c").to_broadcast([128, hi - lo, 128]),
                                    op=ALU.mult)
                            for o, s in _chunks(nk * 128):
                                nc.tensor.matmul(acc[:, o:o + s], lhsT=S_all[:, et * 128:(et + 1) * 128],
                                                 rhs=M[:, o:o + s], start=(ti == 0), stop=(ti == TG - 1))
                        dchunk = degs.tile([128, 1024], BF, tag="dchunk")
                        nc.scalar.copy(dchunk[:, 0:nk * 128], acc[:, 0:nk * 128])
                        tp = degt.tile([128, 1024], BF, tag="dtp", space="PSUM")
                        for j in range(nk):
                            nc.tensor.transpose(tp[:, j * 128:(j + 1) * 128],
                                                dchunk[:, j * 128:(j + 1) * 128], ident_b[:])
                        xs = xT[g][:].rearrange("p (n k) -> p n k", k=NC49)[:, :, k0:k0 + nk]
                        nc.vector.tensor_tensor(
                            xs, xs, tp[:, 0:nk * 128].rearrange("p (j n) -> p n j", j=nk),
                            op=ALU.add)
        dbg("xT0", xT[0][:])
        dbg("xT1", xT[1][:])

        # ---------- rms norm ----------
        # restricted: out[c, (n r)] (29 coeffs). Full: out[c, (q k n)] — quarter-major
        # k-major layout so FFN matmul moving operands are contiguous.
        def rms_norm(nidx, restricted, out_tiles, psp, sbp):
            stride = NR if restricted else NC49
            blocks = RBLK if restricted else [(ks, ks, kc) for (ks, kc) in LBLK]
            for g in range(2):
                sq = sbp.tile([128, W49], BF, tag="sq")
                nc.gpsimd.tensor_tensor(sq[:], xT[g][:], xT[g][:], op=ALU.mult)
                red = sbp.tile([128, 7 * 128], BF, tag="nrm_red")
                with nc.allow_low_precision(reason="bf16 ms-reduce, 0.4% on rms"):
                    for l in range(L_MAX + 1):
                        ks, kc = LBLK[l]
                        nc.vector.tensor_reduce(
                            red[:, l * 128:(l + 1) * 128],
                            sq[:].rearrange("p (n k) -> p n k", k=NC49)[:, :, ks:ks + kc],
                            axis=AX.X, op=ALU.add)
                ms = psp.tile([128, 7 * 128], F32, tag="nrm_ms", space="PSUM")
                for o, s in _chunks(7 * 128):
                    nc.tensor.matmul(ms[:, o:o + s], lhsT=ones128b[:],
                                     rhs=red[:, o:o + s], start=True, stop=True)
                inv = sbp.tile([128, 7 * 128], F32, tag="nrm_inv")
                for l in range(L_MAX + 1):
                    nc.vector.tensor_scalar(inv[:, l * 128:(l + 1) * 128],
                                            ms[:, l * 128:(l + 1) * 128],
                                            float(1.0 / ((2 * l + 1) * C)), EPS,
                                            op0=ALU.mult, op1=ALU.add)
                nc.scalar.activation(inv[:], inv[:], AF.Sqrt)
                nc.vector.reciprocal(inv[:], inv[:])
                for l, (os_, ks, cnt) in enumerate(blocks):
                    if restricted:
                        ov = out_tiles[g][:].rearrange("p (n k) -> p n k", k=stride)[:, :, os_:os_ + cnt]
                        xv = xT[g][:].rearrange("p (n k) -> p n k", k=NC49)[:, :, ks:ks + cnt]
                        iv = inv[:, l * 128:(l + 1) * 128].rearrange("p n -> p n ()") \
                            .to_broadcast([128, 128, cnt])
                    else:
                        ov = out_tiles[g][:].rearrange("p (q k n) -> p q n k", q=4, n=32)[
                            :, :, :, os_:os_ + cnt]
                        xv = xT[g][:].rearrange("p (q n k) -> p q n k", q=4, k=NC49)[
                            :, :, :, ks:ks + cnt]
                        iv = inv[:, l * 128:(l + 1) * 128].rearrange("p (q n) -> p q n ()", q=4) \
                            .to_broadcast([128, 4, 32, cnt])
                    nc.vector.tensor_tensor(ov, xv, iv, op=ALU.mult)
                    nc.scalar.activation(ov, ov, AF.Copy, scale=nwT[:, nidx * 7 + l:nidx * 7 + l + 1])

        # ---------- radial MLP for attention a (runs during the AllGather) ----------
        # transpose-free: layer 1 computed directly in [h1, e] orientation
        # (no PE transpose may overlap an in-flight collective).
        def rad_mlp(a):
            with tc.tile_pool(name=f"rm{a}", bufs=3) as rs, \
                 tc.tile_pool(name=f"rm{a}p", bufs=2, space="PSUM") as rp:
                for et in range(NT):
                    ps = rp.tile([128, 128], F32, tag="mlp1", space="PSUM")
                    for ci in range(5):
                        nc.tensor.matmul(ps[:],
                                         lhsT=radw1c[(a, ci)][:],
                                         rhs=distT[:, ci * EP + et * 128:ci * EP + (et + 1) * 128],
                                         start=(ci == 0), stop=False)
                    nc.tensor.matmul(ps[:], lhsT=radb1[a][:], rhs=ones1[:], start=False, stop=True)
                    s1T = rs.tile([128, 128], BF, tag="s1T")
                    nc.scalar.activation(s1T[:], ps[:], AF.Silu)
                    psr = rp.tile([128, 128], F32, tag="mlp2", space="PSUM")
                    nc.tensor.matmul(psr[:], lhsT=s1T[:], rhs=rad_w2[a][:], start=True, stop=True)
                    nc.vector.tensor_copy(rad_all[:, (a * NT + et) * 128:(a * NT + et + 1) * 128],
                                          psr[:])

        # ---------- attention ----------
        def attention(a, nidx):
            last = (a == 2)
            esA = ExitStack()
            ap_ = esA.enter_context(tc.tile_pool(name=f"at{a}", bufs=1))
            log_all = ap_.tile([128, NT * 8], F32, tag="log_all")
            pooled_sb = ap_.tile([16, W29], BF, tag="pooled_sb", name="pooled_sb") if last else None

            # --- norm + y; AllGather issued as soon as the ys rows land ---
            with tc.tile_pool(name=f"at{a}n", bufs=1) as np_:
                hrT = [np_.tile([128, W29], BF, tag=f"hrT{g}", name=f"hrT{g}") for g in range(2)]
                with tc.tile_pool(name=f"at{a}nn", bufs=1, space="PSUM") as nrmp:
                    rms_norm(nidx, True, hrT, nrmp, np_)
                with tc.tile_pool(name=f"at{a}np", bufs=1, space="PSUM") as ypp, \
                     tc.tile_pool(name=f"at{a}ys", bufs=1) as yss:
                    def yrows(wt, loc, g):
                        hv = hrT[g][:].rearrange("p (n k) -> p k n", k=NR)
                        ysrow = yss.tile([128, W29], BF, tag="ysrow", name="ysrow")
                        yp = ypp.tile([128, 2048], F32, tag="yp", space="PSUM")
                        ci = 0
                        for r0 in range(0, NR, 4):
                            nr = min(4, NR - r0)
                            for j in range(nr):
                                nc.tensor.matmul(yp[:, j * 512:j * 512 + 128],
                                                 lhsT=hv[:, r0 + j, :], rhs=wt[:],
                                                 start=True, stop=True)
                            copy_eng(ci, ysrow[:, r0 * 128:(r0 + nr) * 128]
                                         .rearrange("p (j c) -> p j c", c=128),
                                     yp[:].rearrange("p (j c) -> p j c", c=512)[:, 0:nr, 0:128])
                            ci += 1
                        nc.sync.dma_start(loc[g * 128:(g + 1) * 128, :], ysrow[:])
                        if a == 0 and g == 0 and loc is yt_loc:
                            dbg("ytr0", ysrow[:])

                    for g in range(2):
                        yrows(w_s[a], ys_loc, g)
                    if "ag" not in ablate:
                        nc.gpsimd.collective_compute("AllGather", ALU.bypass, replica_groups=RG,
                                                     ins=[ys_loc[:]], outs=[ys_full[:]])
                    for g in range(2):
                        yrows(w_t[a], yt_loc, g)
            rad_mlp(a)

            # --- edge phase: attention-scope pools, tags shared across groups ---
            mp = esA.enter_context(tc.tile_pool(name=f"at{a}m", bufs=1))
            pp = esA.enter_context(tc.tile_pool(name=f"at{a}p", bufs=1, space="PSUM"))
            for g in range(2):
                agn_g = mp.tile([128, W29], BF, tag="agn", name=f"agn{g}", bufs=1)
                vsb = [mp.tile([128, W29], BF, tag=f"vsb_{ti}", name=f"vsb{g}_{ti}", bufs=1)
                       for ti in range(TG)]
                # local (yt) gathers first — these overlap the AllGather
                for ti in range(TG):
                    et = g * TG + ti
                    nc.gpsimd.indirect_dma_start(
                        out=vsb[ti][:], out_offset=None, in_=yt_loc[:],
                        in_offset=bass.IndirectOffsetOnAxis(ap=idx_dst[:, et:et + 1], axis=0))
                # --- tile loop: remote gather, add+radial, transpose, values, logits ---
                for ti in range(TG):
                    et = g * TG + ti
                    ms2 = mp.tile([128, W29], BF, tag="ms2", bufs=1)
                    nc.gpsimd.indirect_dma_start(
                        out=ms2[:], out_offset=None, in_=ys_full[:],
                        in_offset=bass.IndirectOffsetOnAxis(ap=idx_src[:, et:et + 1], axis=0))
                    eng = nc.vector if ti % 2 == 0 else nc.gpsimd
                    eng.tensor_tensor(vsb[ti][:], vsb[ti][:], ms2[:], op=ALU.add)
                    if a == 0 and g == 0 and ti == 0:
                        dbg("gat0", vsb[0][:])
                    nc.vector.tensor_tensor(
                        vsb[ti][:].rearrange("p (r h) -> p r h", h=128),
                        vsb[ti][:].rearrange("p (r h) -> p r h", h=128),
                        rad_all[:, (a * NT + et) * 128:(a * NT + et + 1) * 128]
                            .rearrange("p h -> p () h").to_broadcast([128, NR, 128]),
                        op=ALU.mult)
                    mtt = mp.tile([128, W29], BF, tag="mtt", bufs=2)
                    accT = pp.tile([128, 1024], BF, tag="accT", space="PSUM", bufs=2)
                    accV = pp.tile([128, 1152], F32, tag="accV", space="PSUM", bufs=1)
                    ci = 0
                    for r0 in range(0, NR, 8):
                        nr = min(8, NR - r0)
                        for j in range(nr):
                            nc.tensor.transpose(accT[:, j * 128:(j + 1) * 128],
                                                vsb[ti][:, (r0 + j) * 128:(r0 + j + 1) * 128],
                                                ident_b[:])
                        nc.vector.tensor_copy(mtt[:, r0 * 128:(r0 + nr) * 128],
                                              accT[:, 0:nr * 128])
                        for j in range(nr):
                            nc.tensor.matmul(accV[:, j * 128:(j + 1) * 128],
                                             lhsT=mtt[:, (r0 + j) * 128:(r0 + j + 1) * 128],
                                             rhs=w_v[a][:], start=True, stop=True)
                        nc.scalar.copy(vsb[ti][:, r0 * 128:(r0 + nr) * 128], accV[:, 0:nr * 128])
                        ci += 1
                    qs = mp.tile([128, 128], BF, tag="qs", bufs=2)
                    nc.scalar.activation(qs[:], mtt[:, 0:128], AF.Silu)
                    nc.vector.tensor_scalar(qs[:], qs[:], avecC[:, a:a + 1], None, op0=ALU.mult)
                    nc.tensor.matmul(accV[:, 1024:1032], lhsT=qs[:], rhs=Hsel[:],
                                     start=True, stop=True)
                    nc.scalar.copy(log_all[:, et * 8:(et + 1) * 8], accV[:, 1024:1032])
                    if a == 0 and et == 0:
                        dbg("msg00", mtt[:])
                        dbg("vsb00", vsb[0][:])

                # --- softmax ---
                lsl = log_all[:, g * TG * 8:(g + 1) * TG * 8]
                pmax = mp.tile([128, 1], F32, tag="pmax", bufs=2)
                nc.vector.tensor_reduce(pmax[:], lsl, axis=AX.X, op=ALU.max)
                sx0 = pp.tile([128, 512], F32, tag="sx", space="PSUM", bufs=2)
                nc.tensor.matmul(sx0[0:1, 0:128], lhsT=pmax[:], rhs=ident[:], start=True, stop=True)
                prow = mp.tile([1, 128], F32, tag="prow", bufs=2)
                nc.scalar.copy(prow[:], sx0[0:1, 0:128])
                mx11 = mp.tile([1, 1], F32, tag="mx11", bufs=2)
                nc.vector.tensor_reduce(mx11[:], prow[:], axis=AX.X, op=ALU.max)
                sx1 = pp.tile([128, 512], F32, tag="sx", space="PSUM", bufs=2)
                nc.tensor.matmul(sx1[:, 0:1], lhsT=ones1f[:], rhs=mx11[:], start=True, stop=True)
                mxcol = mp.tile([128, 1], F32, tag="mxcol", bufs=2)
                nc.scalar.copy(mxcol[:], sx1[:, 0:1])
                exs = mp.tile([128, TG * 8], BF, tag="exs", bufs=2)
                exf = mp.tile([128, TG * 8], F32, tag="exf", bufs=2)
                nc.vector.tensor_scalar(exf[:], lsl, mxcol[:, 0:1], None, op0=ALU.subtract)
                nc.scalar.activation(exs[:], exf[:], AF.Exp)
                dps = pp.tile([128, 512], F32, tag="sx", space="PSUM", bufs=2)
                for ti in range(TG):
                    et = g * TG + ti
                    nc.tensor.matmul(dps[:, 0:8], lhsT=S_all[:, et * 128:(et + 1) * 128],
                                     rhs=exs[:, ti * 8:(ti + 1) * 8],
                                     start=(ti == 0), stop=(ti == TG - 1))
                rden = mp.tile([128, 8], F32, tag="rden", bufs=2)
                nc.vector.tensor_scalar_max(rden[:], dps[:, 0:8], 1e-9)
                nc.vector.reciprocal(rden[:], rden[:])

                # --- scatter: exp-weighted one-hot per head; per-node 1/den folded
                # into the PSUM drain as a per-partition tensor_scalar ---
                agv = agn_g[:].rearrange("p (r h d) -> p h r d", h=8, d=16)
                sh8 = []
                for ti in range(TG):
                    et = g * TG + ti
                    s8 = mp.tile([128, 1024], BF, tag=f"sh8_{ti}", name=f"sh8_{ti}", bufs=1)
                    eng = nc.vector if ti % 2 == 0 else nc.gpsimd
                    eng.tensor_tensor(
                        s8[:].rearrange("p (h n) -> p h n", h=8),
                        S_all[:, et * 128:(et + 1) * 128].rearrange("p n -> p () n")
                            .to_broadcast([128, 8, 128]),
                        exs[:, ti * 8:(ti + 1) * 8].rearrange("p h -> p h ()")
                            .to_broadcast([128, 8, 128]),
                        op=ALU.mult)
                    sh8.append(s8)
                for h2 in range(HEADS):
                    shacc = pp.tile([128, 512], F32, tag="sx", space="PSUM", bufs=2)
                    for ti in range(TG):
                        nc.tensor.matmul(
                            shacc[:, 0:NR * VPH],
                            lhsT=sh8[ti][:, h2 * 128:(h2 + 1) * 128],
                            rhs=vsb[ti][:].rearrange("p (r h d) -> p h r d", h=8, d=16)[:, h2],
                            start=(ti == 0), stop=(ti == TG - 1))
                    if h2 % 2 == 0:
                        nc.vector.tensor_scalar(agv[:, h2],
                                                shacc[:, 0:NR * VPH].rearrange("p (r d) -> p r d", d=16),
                                                rden[:, h2:h2 + 1], None, op0=ALU.mult)
                    else:
                        nc.scalar.activation(agv[:, h2],
                                             shacc[:, 0:NR * VPH].rearrange("p (r d) -> p r d", d=16),
                                             AF.Copy, scale=rden[:, h2:h2 + 1])
                if a == 0 and g == 0:
                    dbg("agg00", agn_g[:])

                # --- project + residual (or latent+pool for last) ---
                if not last:
                    ag = mp.tile([128, W29], BF, tag="ag", bufs=1)
                    ci = 0
                    for r0 in range(0, NR, 8):
                        nr = min(8, NR - r0)
                        acc = pp.tile([128, 1024], BF, tag="accT", space="PSUM", bufs=2)
                        for j in range(nr):
                            nc.tensor.transpose(acc[:, j * 128:(j + 1) * 128],
                                                agn_g[:, (r0 + j) * 128:(r0 + j + 1) * 128],
                                                ident_b[:])
                        copy_eng(ci, ag[:, r0 * 128:(r0 + nr) * 128], acc[:, 0:nr * 128])
                        ci += 1
                    for o, s in _chunks(W29):
                        wacc = pp.tile([128, 512], F32, tag="sx", space="PSUM", bufs=2)
                        nc.tensor.matmul(wacc[:, 0:s], lhsT=w_p[a][:], rhs=ag[:, o:o + s],
                                         start=True, stop=True)
                        r0, r1 = o // 128, (o + s) // 128
                        for (os_, ks, cnt) in RBLK:
                            lo, hi = max(os_, r0), min(os_ + cnt, r1)
                            if lo >= hi:
                                continue
                            xv = xT[g][:].rearrange("p (n k) -> p n k", k=NC49)[
                                :, :, ks + (lo - os_):ks + (hi - os_)]
                            nc.vector.tensor_add(
                                xv, xv,
                                wacc[:, 0:s].rearrange("p (r n) -> p n r", n=128)[
                                    :, :, lo - r0:hi - r0])
                else:
                    for r in range(NR):
                        tpr = pp.tile([128, 1024], BF, tag="accT", space="PSUM", bufs=2)
                        nc.tensor.transpose(tpr[:, 0:128], agn_g[:, r * 128:(r + 1) * 128],
                                            ident_b[:])
                        mt2 = mp.tile([128, 128], BF, tag="qs", bufs=2)
                        nc.scalar.copy(mt2[:], tpr[:, 0:128])
                        vpr = pp.tile([128, 512], F32, tag="sx", space="PSUM", bufs=2)
                        nc.tensor.matmul(vpr[:, 0:128], lhsT=mt2[:], rhs=w_p[a][:],
                                         start=True, stop=True)
                        latr = mp.tile([128, 128], BF, tag="latr", bufs=2)
                        copy_eng(r, latr[:], vpr[:, 0:128])
                        pacc = pp.tile([128, 512], F32, tag="sx", space="PSUM", bufs=2)
                        nc.tensor.matmul(pacc[0:16, 0:128], lhsT=PT[g][:], rhs=latr[:],
                                         start=True, stop=True)
                        if g == 0:
                            nc.scalar.copy(pooled_sb[:, r * 128:(r + 1) * 128], pacc[0:16, 0:128])
                        else:
                            nc.vector.tensor_add(pooled_sb[:, r * 128:(r + 1) * 128],
                                                 pooled_sb[:, r * 128:(r + 1) * 128],
                                                 pacc[0:16, 0:128])

            if a == 0:
                dbg("logits0", log_all[:])
            if last:
                nc.sync.dma_start(pooled_d[:], pooled_sb[:])
            esA.close()

        # ---------- ffn ----------
        def ffn(i, nidx):
            if "ffn" in ablate:
                return
            QW = 32 * NC49      # 1568 cols per quarter
            with tc.tile_pool(name=f"ff{i}", bufs=1) as fp:
                hfull = [fp.tile([128, W49], BF, tag=f"hf{g}", name=f"hf{g}") for g in range(2)]
                with tc.tile_pool(name=f"ff{i}np", bufs=1, space="PSUM") as fnp:
                    rms_norm(nidx, False, hfull, fnp, fp)
                with tc.tile_pool(name=f"ff{i}p", bufs=1, space="PSUM") as ffp, \
                     tc.tile_pool(name=f"ff{i}s", bufs=2) as fs:
                    KCH = [(0, 16), (16, 16), (32, 16), (48, 1)]   # k-chunks (<=512 psum cols)
                    for g in range(2):
                        for q in range(4):
                            # quarter is physically k-major: [c, (k n)] contiguous
                            hkn = hfull[g][:, q * QW:(q + 1) * QW]
                            ops = ffp.tile([128, 2048], F32, tag="ops", space="PSUM")
                            for fc in range(4):
                                h1p = ffp.tile([128, 2048], F32, tag="h1p", space="PSUM")
                                for k0, kn in KCH:
                                    nc.tensor.matmul(h1p[:, k0 * 32:(k0 + kn) * 32],
                                                     lhsT=ffn_w1[(i, fc)][:],
                                                     rhs=hkn[:, k0 * 32:(k0 + kn) * 32],
                                                     start=True, stop=True)
                                sg = fs.tile([128, 32], BF, tag="sg")
                                nc.scalar.activation(sg[:], h1p[:, 0:32], AF.Sigmoid)
                                h1c = fs.tile([128, QW], BF, tag="h1c")
                                nc.scalar.copy(h1c[:], h1p[:, 0:QW])
                                h1g = fs.tile([128, QW], BF, tag="h1g")
                                nc.vector.tensor_tensor(
                                    h1g[:].rearrange("p (k n) -> p k n", n=32),
                                    h1c[:].rearrange("p (k n) -> p k n", n=32),
                                    sg[:].rearrange("p n -> p () n").to_broadcast([128, NC49, 32]),
                                    op=ALU.mult)
                                for o, s in _chunks(QW):
                                    nc.tensor.matmul(ops[:, o:o + s], lhsT=ffn_w2[(i, fc)][:],
                                                     rhs=h1g[:, o:o + s], start=(fc == 0), stop=(fc == 3))
                            xsl = xT[g][:, q * QW:(q + 1) * QW].rearrange("p (n k) -> p n k", k=NC49)
                            nc.vector.tensor_tensor(
                                xsl, xsl,
                                ops[:, 0:QW].rearrange("p (k n) -> p n k", n=32),
                                op=ALU.add)

        attention(0, 0)
        dbg("xT0_a0", xT[0][:])
        ffn(0, 1)
        dbg("xT0_f0", xT[0][:])
        attention(1, 2)
        ffn(1, 3)
        dbg("xT0_l1", xT[0][:])
        dbg("xT1_l1", xT[1][:])
        attention(2, 4)

    nc.compile()
    return nc


_CACHE = {}


def _get_program(meta, debug=()):
    key = (meta["TG"], tuple(n for n, _ in debug), tuple(sorted(ABLATE)))
    if key not in _CACHE:
        _CACHE[key] = build_program(meta, debug, frozenset(ABLATE))
    return _CACHE[key]


DEBUG_OUTS = ()
ABLATE = set()


class _Runner:
    """Caches the jitted shard_map callable for a compiled program."""

    def __init__(self, nc):
        import jax
        from jax.sharding import Mesh, PartitionSpec
        from jax.experimental.shard_map import shard_map
        from concourse.bass2jax import _bass_exec_p, install_neuronx_cc_hook, partition_id_tensor
        install_neuronx_cc_hook()
        self.jax = jax
        pname = nc.partition_id_tensor.name if nc.partition_id_tensor else None
        in_names, out_names, out_avals, zeros = [], [], [], []
        for alloc in nc.m.functions[0].allocations:
            if not isinstance(alloc, mybir.MemoryLocationSet):
                continue
            name = alloc.memorylocations[0].name
            if alloc.kind == "ExternalInput":
                if name != pname:
                    in_names.append(name)
            elif alloc.kind == "ExternalOutput":
                out_names.append(name)
                shp = tuple(alloc.tensor_shape)
                dt = mybir.dt.np(alloc.dtype)
                out_avals.append(jax.core.ShapedArray(shp, dt))
                zeros.append(np.zeros((NCORES * shp[0],) + shp[1:], dt))
        self.in_names, self.out_names, self.zeros = in_names, out_names, zeros
        n_params, n_outs = len(in_names), len(out_names)
        names_all = in_names + out_names + ([pname] if pname else [])

        def _body(*args):
            operands = list(args)
            if pname is not None:
                operands.append(partition_id_tensor())
            return tuple(_bass_exec_p.bind(
                *operands, out_avals=tuple(out_avals), in_names=tuple(names_all),
                out_names=tuple(out_names), lowering_input_output_aliases=(),
                sim_require_finite=True, sim_require_nnan=True, nc=nc))

        devices = jax.devices()[:NCORES]
        self.mesh = Mesh(np.asarray(devices), ("core",))
        self.fn = jax.jit(shard_map(
            _body, mesh=self.mesh,
            in_specs=(PartitionSpec("core"),) * (n_params + n_outs),
            out_specs=(PartitionSpec("core"),) * n_outs, check_rep=False),
            keep_unused=True)

    def stage(self, in_maps):
        from jax.sharding import NamedSharding, PartitionSpec
        sh = NamedSharding(self.mesh, PartitionSpec("core"))
        args = [np.concatenate([np.asarray(m[n]) for m in in_maps], axis=0)
                for n in self.in_names] + list(self.zeros)
        return [self.jax.device_put(a, sh) for a in args]

    def __call__(self, staged):
        return self.fn(*staged)

    def results(self, outs):
        res = [dict() for _ in range(NCORES)]
        for i, n in enumerate(self.out_names):
            arr = np.asarray(outs[i])
            per = arr.reshape(NCORES, arr.shape[0] // NCORES, *arr.shape[1:])
            for c in range(NCORES):
                res[c][n] = per[c]
        return res


_RUNNERS = {}


def get_runner(meta, debug=()):
    key = (meta["TG"], tuple(n for n, _ in debug))
    if key not in _RUNNERS:
        _RUNNERS[key] = _Runner(_get_program(meta, debug))
    return _RUNNERS[key]


def kernel(**inputs):
    meta, in_maps = host_prep(inputs)
    runner = get_runner(meta, DEBUG_OUTS)
    staged = runner.stage(in_maps)
    outs = runner(staged)
    self_results = runner.results(outs)
    pooled = np.zeros((G, NR, 128), np.float32)
    for c in range(NCORES):
        pooled += self_results[c]["pooled"].astype(np.float32).reshape(G, NR, 128)
    out = np.zeros((G, NC49, C), np.float32)
    out[:, RESTRICT_NP, :] = pooled
    kernel.last_results = self_results
    kernel.last_runner = runner
    kernel.last_staged = staged
    return out.reshape(1, -1)
